# revision 11
# baseline (speedup 1.0000x reference)
"""Trainium2 Bass kernel for nn_Block_84198538871557 (dense transformer block).

Sharding: tensor-parallel over 8 NeuronCores.
 - attention: 2 heads per core (w_qkv/ak/av/gating sharded on head dim)
 - proj: sharded on input dim -> partial sums -> chunked ReduceScatter over tokens
 - residual + rmsnorm2: sequence-parallel on each core's 256-token shard
 - MLP: fc1/fc2/mlp_proj sharded on the intermediate dim (704 -> padded 768/core);
   n2^T shards AllGathered; final ReduceScatter over tokens
 - host concatenates the 8 token shards.

Numerics: fp32r (full-speed 4-byte matmul dtype) for all large matmuls;
bf16 only for attention probabilities/values and the mlp down-proj.
Softmax skips max-subtraction (logits bounded, fp32 exp cannot overflow);
causal masking multiplies exp(scores) by a 0/1 mask (identical to -inf mask).
"""
import math
import numpy as np

import concourse.bass as bass
import concourse.mybir as mybir
import concourse.tile as tile
from concourse.bass_utils import run_bass_kernel_spmd

# ---- problem constants (hardcoded per contest contract) ----
B, T, C = 1, 2048, 2048
H, HS = 16, 128
AT, ATP = 20, 32          # adapter length, padded
INTER = 5632
NCORE = 8
HPC = H // NCORE          # heads per core = 2
FPC = HPC * HS            # proj input features per core = 256
IPC0 = INTER // NCORE     # 704
IPC = 768                 # padded intermediate per core (6 x 128)
TSH = T // NCORE          # 256 tokens per core
NRS = 4                   # proj ReduceScatter chunks
FR = T // (NRS * NCORE)   # fragment size 64
EPS = 1e-5
SCALE = 1.0 / math.sqrt(HS)
P = 128
TB = 512                  # attention q-block width
NTB = T // TB             # 4
TBB = 256                 # phase-B t-block width
NTBB = T // TBB           # 8
NCT = C // P              # 16 c-tiles
NTT = T // P              # 16 t-tiles

F32 = mybir.dt.float32
F32R = mybir.dt.float32r
BF16 = mybir.dt.bfloat16
MUL = mybir.AluOpType.mult
ADD = mybir.AluOpType.add
AF = mybir.ActivationFunctionType

_CACHE = {}


def split_excess_waits(nc, max_waits=1):
    """Walrus in this env rejects instructions with >2 attached sem waits.
    Spill excess waits onto NoOps injected right before the instruction."""
    nadd = 0
    for f in nc.m.functions:
        for bb in f.blocks:
            new_insts = []
            for inst in bb.instructions:
                si = inst.sync_info
                waits = list(si.on_wait) if si and si.on_wait else []
                if len(waits) > max_waits:
                    keep = waits[:max_waits]
                    extra = waits[max_waits:]
                    for k in range(0, len(extra), max_waits):
                        chunk = extra[k:k + max_waits]
                        nop = mybir.InstNoOp(
                            name=f"{inst.name}-wsplit{k}", ins=[], outs=[])
                        nop.engine = inst.engine
                        nop.sync_info = mybir.SyncInfo(on_wait=chunk, on_update=[])
                        new_insts.append(nop)
                        nadd += 1
                    si.on_wait = keep
                new_insts.append(inst)
            bb.instructions = new_insts
    return nadd


def build_nc():
    nc = bass.Bass(num_devices=NCORE)
    rg = [list(range(NCORE))]

    # ---- kernel I/O ----
    io = {}
    io["x_in"] = nc.dram_tensor("x", [T, C], F32, kind="ExternalInput")
    io["x_shard"] = nc.dram_tensor("x_shard", [NRS, FR, C], F32, kind="ExternalInput")
    io["cosT"] = nc.dram_tensor("cosT", [HS, T], F32, kind="ExternalInput")
    io["sinT"] = nc.dram_tensor("sinT", [HS, T], F32, kind="ExternalInput")
    io["wqkT"] = nc.dram_tensor("wqkT", [C, 4 * P], F32R, kind="ExternalInput")
    io["wvT"] = nc.dram_tensor("wvT", [C, HPC * HS], F32R, kind="ExternalInput")
    io["akT"] = nc.dram_tensor("akT", [HPC, HS, ATP], F32R, kind="ExternalInput")
    io["av_sc"] = nc.dram_tensor("av_sc", [HPC, ATP, 132], BF16, kind="ExternalInput")
    io["masks"] = nc.dram_tensor("masks", [4, P, TB], BF16, kind="ExternalInput")
    io["rsTm"] = nc.dram_tensor("rsT", [P, P], F32R, kind="ExternalInput")
    io["ident"] = nc.dram_tensor("ident", [P, P], F32R, kind="ExternalInput")
    io["wprojT"] = nc.dram_tensor("wprojT", [FPC, C], F32R, kind="ExternalInput")
    io["fc1T"] = nc.dram_tensor("fc1T", [C, IPC], F32R, kind="ExternalInput")
    io["fc2T"] = nc.dram_tensor("fc2T", [C, IPC], F32R, kind="ExternalInput")
    io["wmT"] = nc.dram_tensor("wmT", [IPC, C], BF16, kind="ExternalInput")
    io["out"] = nc.dram_tensor("out", [TSH, C], F32, kind="ExternalOutput")

    # ---- internal DRAM ----
    io["h_bounce"] = nc.dram_tensor("h_bounce", [T, C], F32)
    io["rs1_out"] = nc.dram_tensor("rs1_out", [NRS, FR, C], F32)
    io["n2t_bounce"] = nc.dram_tensor("n2t_bounce", [C, TSH], F32R)
    io["n2t_all"] = nc.dram_tensor("n2t_all", [NCORE, C, TSH], F32R,
                                   addr_space="Shared")
    io["mlp_bounce"] = nc.dram_tensor("mlp_bounce", [NCORE * TSH, C], F32)
    io["rs2_out"] = nc.dram_tensor("rs2_out", [TSH, C], F32)

    with tile.TileContext(nc) as tc:
        _build_body(nc, tc, rg, io)
    return nc


def _build_body(nc, tc, rg, io):
    x_in, x_shard = io["x_in"], io["x_shard"]
    cosT, sinT = io["cosT"], io["sinT"]
    wqkT, wvT, akT, av_sc = io["wqkT"], io["wvT"], io["akT"], io["av_sc"]
    masks, rsTm, ident = io["masks"], io["rsTm"], io["ident"]
    wprojT, fc1T, fc2T, wmT = io["wprojT"], io["fc1T"], io["fc2T"], io["wmT"]
    out = io["out"]
    h_bounce, rs1_out = io["h_bounce"], io["rs1_out"]
    n2t_bounce, n2t_all = io["n2t_bounce"], io["n2t_all"]
    mlp_bounce, rs2_out = io["mlp_bounce"], io["rs2_out"]

    with (
        tc.tile_pool(name="const", bufs=1) as const,
        tc.tile_pool(name="xh", bufs=1) as xh_pool,
    ):
        # ---- constants in SBUF (small, live whole kernel) ----
        id_sb = const.tile([P, P], F32R)
        nc.sync.dma_start(id_sb[:], ident[:, :])
        mask_sb = const.tile([P, 4, TB], BF16)
        nc.sync.dma_start(mask_sb[:], masks.ap().rearrange("m p q -> p m q"))
        akT_sb = const.tile([P, HPC, ATP], F32R)
        nc.sync.dma_start(akT_sb[:], akT.ap().rearrange("h p a -> p h a"))
        av_sb = const.tile([ATP, HPC, 132], BF16)
        nc.sync.dma_start(av_sb[:], av_sc.ap().rearrange("h a v -> a h v"))
        eps_sb = const.tile([P, 1], F32)
        nc.vector.memset(eps_sb[:], EPS)
        xh_sb = [xh_pool.tile([P, C], F32, tag=f"xh{s}", name=f"xh{s}")
                 for s in range(2)]

        with (
            tc.tile_pool(name="qk", bufs=1) as qk_pool,
            tc.tile_pool(name="v", bufs=1) as v_pool,
        ):
            qkT_sb = [qk_pool.tile([P, T], F32R, tag=f"qk{f}", name=f"qk{f}")
                      for f in range(4)]
            v_sb = [v_pool.tile([P, NTT, 132], BF16, tag=f"v{h}", name=f"v{h}")
                    for h in range(HPC)]
            for h in range(HPC):
                nc.vector.memset(v_sb[h][:, :, 128:132], 0.0)
                nc.vector.memset(v_sb[h][:, :, 128:129], 1.0)

            # ---------- phase A+B: norm1 -> n1T -> qkv(+rope) + v ----------
            with (
                tc.tile_pool(name="w1", bufs=1) as w1p,
                tc.tile_pool(name="cs", bufs=3) as csp,
                tc.tile_pool(name="ab_big", bufs=2) as abbig,
                tc.tile_pool(name="ab_sm", bufs=2) as absm,
                tc.tile_pool(name="n1t", bufs=2) as n1tp,
                tc.tile_pool(name="ab_ps", bufs=2, space="PSUM") as abps,
                tc.tile_pool(name="tr_ps", bufs=2, space="PSUM") as trps,
            ):
                rsT_sb = w1p.tile([P, P], F32R)
                nc.sync.dma_start(rsT_sb[:], rsTm[:, :])
                wqk_sb = w1p.tile([P, NCT, 4 * P], F32R)
                nc.sync.dma_start(wqk_sb[:],
                                  wqkT.ap().rearrange("(o p) n -> p o n", p=P))
                wv_sb = w1p.tile([P, NCT, HPC * HS], F32R)
                nc.sync.dma_start(wv_sb[:],
                                  wvT.ap().rearrange("(o p) n -> p o n", p=P))
                for tb in range(NTBB):
                    t0 = tb * TBB
                    n1t = n1tp.tile([P, NCT, TBB], F32R, tag="n1t", name="n1t")
                    for tt2 in range(TBB // P):
                        tt = tb * (TBB // P) + tt2
                        x_sb = abbig.tile([P, C], F32, tag="xin", name="xin")
                        nc.sync.dma_start(x_sb[:], x_in[tt * P:(tt + 1) * P, :])
                        scr = abbig.tile([P, C], BF16, tag="scr", name="scr")
                        ssq = absm.tile([P, 1], F32, tag="ssq", name="ssq")
                        nc.scalar.activation(scr[:], x_sb[:], AF.Square,
                                             accum_out=ssq[:])
                        srt = absm.tile([P, 1], F32, tag="srt", name="srt")
                        nc.scalar.activation(srt[:], ssq[:], AF.Sqrt,
                                             bias=eps_sb[:], scale=1.0 / C)
                        rsv = absm.tile([P, 1], F32, tag="rsv", name="rsv")
                        nc.vector.reciprocal(rsv[:], srt[:])
                        n1f = abbig.tile([P, C], F32R, tag="n1f", name="n1f")
                        nc.vector.tensor_scalar_mul(n1f[:], x_sb[:], rsv[:])
                        for ic in range(NCT):
                            pt = trps.tile([P, P], F32R, tag="trp", name="trp")
                            nc.tensor.transpose(
                                pt[:], n1f[:, ic * P:(ic + 1) * P], id_sb[:])
                            if (ic % 4) != 3:
                                nc.vector.tensor_copy(
                                    out=n1t[:, ic, tt2 * P:(tt2 + 1) * P],
                                    in_=pt[:])
                            else:
                                nc.scalar.copy(
                                    out=n1t[:, ic, tt2 * P:(tt2 + 1) * P],
                                    in_=pt[:])
                    # q/k transposed with rope
                    cs = csp.tile([P, 2, TBB], F32, tag="cs", name="cs")
                    nc.sync.dma_start(cs[:, 0, :], cosT[:, t0:t0 + TBB])
                    nc.sync.dma_start(cs[:, 1, :], sinT[:, t0:t0 + TBB])
                    for f in range(4):
                        ps = abps.tile([P, TBB], F32, tag="qkps", name="qkps")
                        for ic in range(NCT):
                            nc.tensor.matmul(
                                ps[:], wqk_sb[:, ic, f * P:(f + 1) * P],
                                n1t[:, ic, :],
                                start=(ic == 0), stop=(ic == NCT - 1))
                        raw = absm.tile([P, TBB], F32R, tag="rraw", name="rraw")
                        nc.scalar.copy(out=raw[:], in_=ps[:])
                        rot = trps.tile([P, TBB], F32, tag="rotps", name="rotps")
                        nc.tensor.matmul(rot[:], rsT_sb[:], raw[:],
                                         start=True, stop=True)
                        t1 = absm.tile([P, TBB], F32, tag="rt1", name="rt1")
                        nc.vector.tensor_tensor(
                            out=t1[:], in0=raw[:], in1=cs[:, 0, :], op=MUL)
                        t2 = absm.tile([P, TBB], F32, tag="rt2", name="rt2")
                        nc.vector.tensor_tensor(
                            out=t2[:], in0=rot[:], in1=cs[:, 1, :], op=MUL)
                        nc.vector.tensor_add(
                            out=qkT_sb[f][:, t0:t0 + TBB], in0=t1[:], in1=t2[:])
                    # v natural
                    for tt2 in range(TBB // P):
                        tt = tb * (TBB // P) + tt2
                        psv = abps.tile([P, HPC * HS], F32, tag="vps", name="vps")
                        for ic in range(NCT):
                            nc.tensor.matmul(
                                psv[:], n1t[:, ic, tt2 * P:(tt2 + 1) * P],
                                wv_sb[:, ic, :],
                                start=(ic == 0), stop=(ic == NCT - 1))
                        for h in range(HPC):
                            nc.vector.tensor_copy(
                                out=v_sb[h][:, tt, 0:128],
                                in_=psv[:, h * HS:(h + 1) * HS])

            # ---------- phase C+D: attention + proj + chunked RS ----------
            with (
                tc.tile_pool(name="yt", bufs=1) as yt_pool,
                tc.tile_pool(name="wp", bufs=1) as wp_pool,
                tc.tile_pool(name="pt_pool", bufs=18) as ptp,
                tc.tile_pool(name="at_sb", bufs=4) as atsb,
                tc.tile_pool(name="s_ps", bufs=2, space="PSUM") as sps,
                tc.tile_pool(name="y_ps", bufs=2, space="PSUM") as yps,
                tc.tile_pool(name="y2_ps", bufs=1, space="PSUM") as y2ps,
                tc.tile_pool(name="d_ps", bufs=1, space="PSUM") as dps,
            ):
                yT_sb = [yt_pool.tile([P, T], F32R, tag=f"yt{h}", name=f"yt{h}")
                         for h in range(HPC)]
                wproj_sb = wp_pool.tile([P, FPC // P, C], F32R)
                nc.sync.dma_start(wproj_sb[:],
                                  wprojT.ap().rearrange("(o p) n -> p o n", p=P))
                for qb in range(NTB):
                    for h in range(HPC):
                        qT = qkT_sb[2 * h]
                        kT = qkT_sb[2 * h + 1]
                        nkt = 4 * qb + 4
                        pts = []
                        for kt in range(nkt):
                            ps_s = sps.tile([P, TB], F32, tag="sps", name="sps")
                            nc.tensor.matmul(
                                ps_s[:], kT[:, kt * P:(kt + 1) * P],
                                qT[:, qb * TB:(qb + 1) * TB],
                                start=True, stop=True)
                            pT = ptp.tile([P, TB], BF16, tag="pT", name="pT")
                            nc.scalar.activation(pT[:], ps_s[:], AF.Exp,
                                                 scale=SCALE)
                            if kt >= 4 * qb:
                                nc.vector.tensor_tensor(
                                    out=pT[:], in0=pT[:],
                                    in1=mask_sb[:, kt - 4 * qb, :], op=MUL)
                            pts.append(pT)
                        ps2 = y2ps.tile([ATP, TB], F32, tag="s2ps", name="s2ps")
                        nc.tensor.matmul(ps2[:], akT_sb[:, h, :],
                                         qT[:, qb * TB:(qb + 1) * TB],
                                         start=True, stop=True)
                        p2T = atsb.tile([ATP, TB], BF16, tag="p2T", name="p2T")
                        nc.scalar.activation(p2T[:], ps2[:], AF.Exp, scale=SCALE)
                        for j2 in range(4):
                            gq = 4 * qb + j2
                            ps_y = yps.tile([P, 132], F32, tag="yps", name="yps")
                            for kt in range(gq + 1):
                                nc.tensor.matmul(
                                    ps_y[:], pts[kt][:, j2 * P:(j2 + 1) * P],
                                    v_sb[h][:, kt, :],
                                    start=(kt == 0), stop=(kt == gq))
                            ps_y2 = y2ps.tile([P, 132], F32, tag="y2ps",
                                              name="y2ps")
                            nc.tensor.matmul(
                                ps_y2[:], p2T[:, j2 * P:(j2 + 1) * P],
                                av_sb[:, h, :], start=True, stop=True)
                            r1 = atsb.tile([P, 1], F32, tag="r1", name="r1")
                            nc.vector.reciprocal(r1[:], ps_y[:, 128:129])
                            r2 = atsb.tile([P, 1], F32, tag="r2", name="r2")
                            nc.vector.reciprocal(r2[:], ps_y2[:, 128:129])
                            t1 = atsb.tile([P, P], F32, tag="yc1", name="yc1")
                            nc.vector.tensor_scalar_mul(
                                t1[:], ps_y[:, 0:128], r1[:])
                            ycomb = atsb.tile([P, P], F32R, tag="yc2", name="yc2")
                            nc.vector.scalar_tensor_tensor(
                                out=ycomb[:], in0=ps_y2[:, 0:128], scalar=r2[:],
                                in1=t1[:], op0=MUL, op1=ADD)
                            ps_t = dps.tile([P, P], F32R, tag="ytp", name="ytp")
                            nc.tensor.transpose(ps_t[:], ycomb[:], id_sb[:])
                            if gq % 2 == 0:
                                nc.vector.tensor_copy(
                                    out=yT_sb[h][:, gq * P:(gq + 1) * P],
                                    in_=ps_t[:])
                            else:
                                nc.scalar.copy(
                                    out=yT_sb[h][:, gq * P:(gq + 1) * P],
                                    in_=ps_t[:])
                    # proj for this q-block's 4 t-tiles, then RS chunk qb
                    for tt4 in range(4):
                        tt = qb * 4 + tt4
                        for co in range(4):
                            ps_h = dps.tile([P, TB], F32, tag="hps", name="hps")
                            nc.tensor.matmul(
                                ps_h[:], yT_sb[0][:, tt * P:(tt + 1) * P],
                                wproj_sb[:, 0, co * TB:(co + 1) * TB],
                                start=True, stop=False)
                            nc.tensor.matmul(
                                ps_h[:], yT_sb[1][:, tt * P:(tt + 1) * P],
                                wproj_sb[:, 1, co * TB:(co + 1) * TB],
                                start=False, stop=True)
                            h_ev = atsb.tile([P, TB], F32, tag="hev", name="hev")
                            nc.vector.tensor_copy(out=h_ev[:], in_=ps_h[:])
                            nc.sync.dma_start(
                                h_bounce[tt * P:(tt + 1) * P,
                                         co * TB:(co + 1) * TB],
                                h_ev[:])
                    nc.gpsimd.collective_compute(
                        "ReduceScatter", ADD, replica_groups=rg,
                        ins=[h_bounce[qb * TB:(qb + 1) * TB, :]],
                        outs=[rs1_out[qb]],
                    )

        # ---------- phase E: residual + norm2 -> n2T -> AllGather ----------
        with (
            tc.tile_pool(name="e_sb", bufs=2) as esb,
            tc.tile_pool(name="e_sm", bufs=2) as esm,
            tc.tile_pool(name="n2t_sb", bufs=1) as n2tp,
            tc.tile_pool(name="e_ps", bufs=3, space="PSUM") as eps_pool,
        ):
            n2t = n2tp.tile([P, NCT, TSH], F32R)
            for st in range(2):
                r1sb = esb.tile([P, C], F32, tag="r1sb", name="r1sb")
                nc.sync.dma_start(
                    r1sb[:],
                    rs1_out[2 * st:2 * st + 2].rearrange("b f c -> (b f) c"))
                xs_sb = esb.tile([P, C], F32, tag="xssb", name="xssb")
                nc.sync.dma_start(
                    xs_sb[:],
                    x_shard[2 * st:2 * st + 2].rearrange("b f c -> (b f) c"))
                nc.vector.tensor_add(out=xh_sb[st][:], in0=r1sb[:], in1=xs_sb[:])
                scr = esb.tile([P, C], BF16, tag="scr2", name="scr2")
                ssq = esm.tile([P, 1], F32, tag="ssq2", name="ssq2")
                nc.scalar.activation(scr[:], xh_sb[st][:], AF.Square,
                                     accum_out=ssq[:])
                srt = esm.tile([P, 1], F32, tag="srt2", name="srt2")
                nc.scalar.activation(srt[:], ssq[:], AF.Sqrt,
                                     bias=eps_sb[:], scale=1.0 / C)
                rsv = esm.tile([P, 1], F32, tag="rsv2", name="rsv2")
                nc.vector.reciprocal(rsv[:], srt[:])
                n2f = esb.tile([P, C], F32R, tag="n2f", name="n2f")
                nc.vector.tensor_scalar_mul(n2f[:], xh_sb[st][:], rsv[:])
                for ic in range(NCT):
                    pt = eps_pool.tile([P, P], F32R, tag="etp", name="etp")
                    nc.tensor.transpose(
                        pt[:], n2f[:, ic * P:(ic + 1) * P], id_sb[:])
                    if ic % 2 == 0:
                        nc.vector.tensor_copy(
                            out=n2t[:, ic, st * P:(st + 1) * P], in_=pt[:])
                    else:
                        nc.scalar.copy(
                            out=n2t[:, ic, st * P:(st + 1) * P], in_=pt[:])
            nc.sync.dma_start(
                n2t_bounce.ap().rearrange("(o p) t -> p o t", p=P), n2t[:])
            nc.gpsimd.collective_compute(
                "AllGather", mybir.AluOpType.bypass, replica_groups=rg,
                ins=[n2t_bounce.ap()], outs=[n2t_all.ap()],
            )

        # ---------- phase F: MLP + RS2 + final residual ----------
        with (
            tc.tile_pool(name="fc", bufs=1) as fcp,
            tc.tile_pool(name="f_sb", bufs=2) as fsb,
            tc.tile_pool(name="tail", bufs=1) as tailp,
            tc.tile_pool(name="n2r", bufs=2) as n2rp,
            tc.tile_pool(name="g_sb", bufs=8) as gsb,
            tc.tile_pool(name="u_ps", bufs=2, space="PSUM") as ups,
            tc.tile_pool(name="m_ps", bufs=2, space="PSUM") as mps,
        ):
            fc1_sb = fcp.tile([P, NCT, IPC], F32R)
            nc.sync.dma_start(fc1_sb[:],
                              fc1T.ap().rearrange("(o p) n -> p o n", p=P))
            fc2_sb = fcp.tile([P, NCT, IPC], F32R)
            nc.sync.dma_start(fc2_sb[:],
                              fc2T.ap().rearrange("(o p) n -> p o n", p=P))
            wm_sb = fcp.tile([P, IPC // P, C], BF16)
            nc.sync.dma_start(wm_sb[:],
                              wmT.ap().rearrange("(o p) n -> p o n", p=P))
            NI = IPC // P  # 6
            for r in range(NCORE):
                n2r = n2rp.tile([P, NCT, TSH], F32R, tag="n2r", name="n2r")
                nc.sync.dma_start(
                    n2r[:], n2t_all[r].rearrange("(o p) t -> p o t", p=P))
                gts = []
                for i in range(NI):
                    ps1 = ups.tile([P, TSH], F32, tag="u1ps", name="u1ps")
                    for ic in range(NCT):
                        nc.tensor.matmul(
                            ps1[:], fc1_sb[:, ic, i * P:(i + 1) * P],
                            n2r[:, ic, :],
                            start=(ic == 0), stop=(ic == NCT - 1))
                    ps2 = ups.tile([P, TSH], F32, tag="u2ps", name="u2ps")
                    for ic in range(NCT):
                        nc.tensor.matmul(
                            ps2[:], fc2_sb[:, ic, i * P:(i + 1) * P],
                            n2r[:, ic, :],
                            start=(ic == 0), stop=(ic == NCT - 1))
                    s_sb = fsb.tile([P, TSH], F32, tag="silu", name="silu")
                    nc.scalar.activation(s_sb[:], ps1[:], AF.Sigmoid)
                    u1s = fsb.tile([P, TSH], F32, tag="u1s", name="u1s")
                    nc.vector.tensor_tensor(out=u1s[:], in0=ps1[:], in1=s_sb[:],
                                            op=MUL)
                    gt = gsb.tile([P, TSH], BF16, tag="gt", name="gt")
                    nc.vector.tensor_tensor(out=gt[:], in0=u1s[:], in1=ps2[:],
                                            op=MUL)
                    gts.append(gt)
                for ts in range(2):
                    for co in range(4):
                        ps_m = mps.tile([P, TB], F32, tag="mps", name="mps")
                        for i in range(NI):
                            nc.tensor.matmul(
                                ps_m[:], gts[i][:, ts * P:(ts + 1) * P],
                                wm_sb[:, i, co * TB:(co + 1) * TB],
                                start=(i == 0), stop=(i == NI - 1))
                        m_ev = fsb.tile([P, TB], F32, tag="mev", name="mev")
                        nc.vector.tensor_copy(out=m_ev[:], in_=ps_m[:])
                        nc.sync.dma_start(
                            mlp_bounce[r * TSH + ts * P: r * TSH + (ts + 1) * P,
                                       co * TB:(co + 1) * TB],
                            m_ev[:])
            nc.gpsimd.collective_compute(
                "ReduceScatter", ADD, replica_groups=rg,
                ins=[mlp_bounce.ap()], outs=[rs2_out.ap()],
            )
            for st in range(2):
                r2sb = tailp.tile([P, C], F32, tag="r2sb", name="r2sb")
                nc.sync.dma_start(r2sb[:], rs2_out[st * P:(st + 1) * P, :])
                o_sb = tailp.tile([P, C], F32, tag="osb", name="osb")
                nc.vector.tensor_add(out=o_sb[:], in0=r2sb[:], in1=xh_sb[st][:])
                nc.sync.dma_start(out[st * P:(st + 1) * P, :], o_sb[:])


# ================= host side =================

def _prep_inputs(x, cos, sin, ak, av, w_qkv, w_proj, gating,
                 norm1_w, norm2_w, w_fc1, w_fc2, w_mlp_proj):
    """Build per-core input maps. All host work in fp32 numpy."""
    import ml_dtypes
    x2 = np.ascontiguousarray(np.asarray(x, np.float32).reshape(T, C))
    cosT = np.ascontiguousarray(np.asarray(cos, np.float32).T)
    sinT = np.ascontiguousarray(np.asarray(sin, np.float32).T)
    w_qkv = np.asarray(w_qkv, np.float32) * np.asarray(norm1_w, np.float32)[None, :]
    w_proj = np.asarray(w_proj, np.float32)
    ak = np.asarray(ak, np.float32)
    av = np.asarray(av, np.float32)
    gating = np.asarray(gating, np.float32)
    fc1w = np.asarray(w_fc1, np.float32) * np.asarray(norm2_w, np.float32)[None, :]
    fc2w = np.asarray(w_fc2, np.float32) * np.asarray(norm2_w, np.float32)[None, :]
    wm = np.asarray(w_mlp_proj, np.float32)

    ident = np.eye(P, dtype=np.float32)
    rsT = np.zeros((P, P), np.float32)
    for d in range(64):
        rsT[d + 64, d] = -1.0       # rot[d] = -x[d+64], d < 64
        rsT[d, d + 64] = 1.0        # rot[d] = x[d-64],  d >= 64
    q_ar = np.arange(TB)
    mk = np.zeros((4, P, TB), np.float32)
    for r in range(4):
        kp = 128 * r + np.arange(P)
        mk[r] = (kp[:, None] <= q_ar[None, :]).astype(np.float32)
    mk = mk.astype(ml_dtypes.bfloat16)

    in_maps = []
    for c in range(NCORE):
        heads = [HPC * c + i for i in range(HPC)]
        cols = []
        for h in heads:
            cols.append(w_qkv[384 * h:384 * h + 128])          # q
            cols.append(w_qkv[384 * h + 128:384 * h + 256])    # k
        wqkTc = np.ascontiguousarray(np.concatenate(cols, axis=0).T)
        vs = [w_qkv[384 * h + 256:384 * h + 384] for h in heads]
        wvTc = np.ascontiguousarray(np.concatenate(vs, axis=0).T)
        akTc = np.zeros((HPC, HS, ATP), np.float32)
        av_c = np.zeros((HPC, ATP, 132), np.float32)
        for i, h in enumerate(heads):
            akTc[i, :, :AT] = ak[0, h].T
            av_c[i, :AT, :128] = gating[0, 0, h, 0] * av[0, h]
            av_c[i, :AT, 128] = 1.0
        av_c = av_c.astype(ml_dtypes.bfloat16)
        wprojTc = np.ascontiguousarray(w_proj[:, FPC * c:FPC * (c + 1)].T)
        r0 = IPC0 * c
        f1 = np.zeros((C, IPC), np.float32)
        f1[:, :IPC0] = fc1w[r0:r0 + IPC0].T
        f2 = np.zeros((C, IPC), np.float32)
        f2[:, :IPC0] = fc2w[r0:r0 + IPC0].T
        wmTc = np.zeros((IPC, C), np.float32)
        wmTc[:IPC0] = wm[:, r0:r0 + IPC0].T
        wmTc = wmTc.astype(ml_dtypes.bfloat16)
        xsh = np.ascontiguousarray(x2.reshape(NRS, NCORE, FR, C)[:, c])
        in_maps.append(dict(
            x=x2, x_shard=xsh, cosT=cosT, sinT=sinT,
            wqkT=wqkTc, wvT=wvTc, akT=akTc, av_sc=av_c, masks=mk,
            rsT=rsT, ident=ident, wprojT=wprojTc,
            fc1T=f1, fc2T=f2, wmT=wmTc,
        ))
    return in_maps


def kernel(x, cos, sin, mask, ak, av, w_qkv, w_proj, gating,
           norm1_w, norm2_w, w_fc1, w_fc2, w_mlp_proj):
    if "nc" not in _CACHE:
        nc = build_nc()
        split_excess_waits(nc)
        _CACHE["nc"] = nc
    nc = _CACHE["nc"]
    in_maps = _prep_inputs(x, cos, sin, ak, av, w_qkv, w_proj, gating,
                           norm1_w, norm2_w, w_fc1, w_fc2, w_mlp_proj)
    res = run_bass_kernel_spmd(nc, in_maps, list(range(NCORE)))
    full = np.empty((NRS, NCORE, FR, C), np.float32)
    for c in range(NCORE):
        full[:, c] = res.results[c]["out"].reshape(NRS, FR, C)
    return full.reshape(1, T, C)
